# revision 2
# baseline (speedup 1.0000x reference)
"""GQA kernel for Trainium2, 8 NeuronCores, tensor-parallel over heads.

Problem: B=1, T=2048, C=4096, 32 q-heads, 16 kv-heads, head_dim=128,
scale = 1/sqrt(32), causal. q head H uses kv head H%16.

Sharding (no collectives needed): core c owns q-heads
{2c, 2c+1, 2c+16, 2c+17} and kv-heads {2c, 2c+1}. Each output column
block depends only on its own head, so the full output is a host-side
concat of per-core column slices.

Per-core kernel (all matmuls bf16, fp32 PSUM accumulation):
  xT resident in SBUF as [C=4096 (32 chunks of 128 part), T=2048].
  qT[h] = Wq_h @ xT    -> [128 (D), 2048 (T)]   (4 heads)
  kT[kv] = Wk_kv @ xT  -> [128 (D), 2048 (T)]   (2 kv heads)
  v[kv]  = x @ Wv_kv^T -> [2048 (T part), 128+1] (ones col for row sums)
  S^T tile = kT_chunk^T @ qT_block -> [128 Tk, 512 Tq] PSUM
  P^T = exp(SCALE * S^T) (ACT, no max subtraction -- logits are O(+-16),
        safe in fp32/bf16), causal mask via {0,1} multiply on diag tiles.
  out[Tq,128] (+ row-sum col) = sum_j P^T_j.T @ v_j  (PSUM accum)
  out normalized by reciprocal(row sum) (DVE), DMA'd out fp32.
"""

import numpy as np
import ml_dtypes

BF16 = ml_dtypes.bfloat16
T = 2048
C = 4096
D = 128
N_HEADS = 32
N_KV = 16
SCALE = float(1.0 / np.sqrt(np.float32(N_HEADS)))
KC = C // 128          # 32 contraction chunks
NQH = 4                # local q heads per core
NKV = 2                # local kv heads per core
NT = T // 128          # 16 token tiles
VROW = D + 1           # 129: v with ones column
N_CORES = 8

_prog_cache = {}


def _build_program():
    if "nc" in _prog_cache:
        return _prog_cache["nc"]
    import concourse.bass as bass
    import concourse.tile as tile
    from concourse import bacc, mybir

    dt = mybir.dt
    f32 = dt.float32
    bf16 = dt.bfloat16
    EXP = mybir.ActivationFunctionType.Exp

    nc = bacc.Bacc("TRN2", target_bir_lowering=False, debug=False,
                   num_devices=N_CORES)

    xT_d = nc.dram_tensor("xT", [128, KC * T], bf16, kind="ExternalInput").ap()
    wq_d = nc.dram_tensor("wq", [NQH, 128, C], bf16, kind="ExternalInput").ap()
    wk_d = nc.dram_tensor("wk", [NKV, 128, C], bf16, kind="ExternalInput").ap()
    wv_d = nc.dram_tensor("wv", [NKV, 128, C], bf16, kind="ExternalInput").ap()
    mask_d = nc.dram_tensor("masks", [128, 4 * 512], bf16,
                            kind="ExternalInput").ap()
    out_d = nc.dram_tensor("out", [T, NQH * D], f32, kind="ExternalOutput").ap()

    with tile.TileContext(nc) as tc:
        with (
            tc.tile_pool(name="persist", bufs=1) as persist,
            tc.tile_pool(name="xpool", bufs=1) as xpool,
            tc.tile_pool(name="wpool", bufs=2) as wpool,
            tc.tile_pool(name="ptpool", bufs=3) as ptpool,
            tc.tile_pool(name="opool", bufs=4) as opool,
            tc.tile_pool(name="recpool", bufs=4) as recpool,
            tc.tile_pool(name="psum", bufs=8, space=bass.MemorySpace.PSUM) as psum,
        ):
            mask_sb = persist.tile([128, 4 * 512], bf16, name="mask_sb",
                                   tag="mask_sb")
            nc.sync.dma_start(out=mask_sb[:], in_=mask_d[:])

            qt = persist.tile([128, NQH * T], bf16, name="qt", tag="qt")
            kt = persist.tile([128, NKV * T], bf16, name="kt", tag="kt")
            vt = persist.tile([128, NKV * NT * VROW], bf16, name="vt", tag="vt")

            # ones columns of v (row-sum trick)
            for i in range(NKV * NT):
                nc.vector.memset(vt[:, i * VROW + D: (i + 1) * VROW], 1.0)

            # xT in 4 groups of 8 chunks so early matmuls start before the
            # full 16MB lands
            xts = []
            for g in range(4):
                xt = xpool.tile([128, 8 * T], bf16, name=f"xt{g}", tag=f"xt{g}")
                nc.sync.dma_start(out=xt[:], in_=xT_d[:, g * 8 * T:(g + 1) * 8 * T])
                xts.append(xt)

            def xs(kc, lo, size):
                g, off = divmod(kc, 8)
                return xts[g][:, off * T + lo: off * T + lo + size]

            def proj_dt(w_src, idx, dest, dbase):
                """[D, T] projection strip: dest[:, dbase:dbase+T] = W @ xT."""
                w = wpool.tile([128, C], bf16, name=f"w_{dest.name}_{idx}",
                               tag="w")
                nc.sync.dma_start(out=w[:], in_=w_src[idx])
                for t4 in range(4):
                    ps = psum.tile([128, 512], f32, name=f"ps_{dest.name}_{idx}_{t4}",
                                   tag="ps")
                    for kc in range(KC):
                        nc.tensor.matmul(
                            ps[:],
                            lhsT=w[:, kc * 128:(kc + 1) * 128],
                            rhs=xs(kc, t4 * 512, 512),
                            start=(kc == 0), stop=(kc == KC - 1),
                        )
                    nc.vector.tensor_copy(
                        out=dest[:, dbase + t4 * 512: dbase + (t4 + 1) * 512],
                        in_=ps[:])

            def proj_v(kv):
                """[T, D] projection: v rows for kv head, into vt."""
                w = wpool.tile([128, C], bf16, name=f"w_v_{kv}", tag="w")
                nc.sync.dma_start(out=w[:], in_=wv_d[kv])
                for m in range(NT):
                    ps = psum.tile([128, 512], f32, name=f"ps_v_{kv}_{m}",
                                   tag="ps")
                    for kc in range(KC):
                        nc.tensor.matmul(
                            ps[:, 0:128],
                            lhsT=xs(kc, m * 128, 128),
                            rhs=w[:, kc * 128:(kc + 1) * 128],
                            start=(kc == 0), stop=(kc == KC - 1),
                        )
                    nc.vector.tensor_copy(
                        out=vt[:, (kv * NT + m) * VROW: (kv * NT + m) * VROW + D],
                        in_=ps[:, 0:128])

            # order: q0 k0 v0 | q1 k1 v1 | q2 q3 -- lets attention on head 0/1
            # start while later projections still run
            proj_dt(wq_d, 0, qt, 0)
            proj_dt(wk_d, 0, kt, 0)
            proj_v(0)
            proj_dt(wq_d, 1, qt, T)
            proj_dt(wk_d, 1, kt, T)
            proj_v(1)
            proj_dt(wq_d, 2, qt, 2 * T)
            proj_dt(wq_d, 3, qt, 3 * T)

            for h in range(NQH):
                kv = h % 2
                for b in range(4):  # Tq blocks of 512
                    pvs = []
                    for s in range(4):
                        pv = psum.tile([128, 512], f32, name=f"pv_{h}_{b}_{s}",
                                       tag="ps")
                        pvs.append(pv)
                    for j in range(4 * b + 4):  # Tk tiles of 128
                        sp = psum.tile([128, 512], f32, name=f"sp_{h}_{b}_{j}",
                                       tag="ps")
                        nc.tensor.matmul(
                            sp[:],
                            lhsT=kt[:, kv * T + j * 128: kv * T + (j + 1) * 128],
                            rhs=qt[:, h * T + b * 512: h * T + (b + 1) * 512],
                            start=True, stop=True,
                        )
                        pt = ptpool.tile([128, 512], bf16, name=f"pt_{h}_{b}_{j}",
                                         tag="pt")
                        nc.scalar.activation(pt[:], sp[:], EXP, scale=SCALE)
                        r = j - 4 * b
                        if r >= 0:
                            nc.vector.tensor_mul(
                                pt[:], pt[:], mask_sb[:, r * 512:(r + 1) * 512])
                        vsl = vt[:, (kv * NT + j) * VROW: (kv * NT + j + 1) * VROW]
                        for s in range(max(0, r), 4):
                            nc.tensor.matmul(
                                pvs[s][:, 0:VROW],
                                lhsT=pt[:, s * 128:(s + 1) * 128],
                                rhs=vsl,
                                start=(j == 0), stop=(j == 4 * b + s),
                            )
                    for s in range(4):
                        rec = recpool.tile([128, 1], f32, name=f"rec_{h}_{b}_{s}",
                                           tag="rec")
                        nc.vector.reciprocal(rec[:], pvs[s][:, D:D + 1])
                        ot = opool.tile([128, 128], f32, name=f"ot_{h}_{b}_{s}",
                                        tag="ot")
                        nc.vector.tensor_scalar_mul(ot[:], pvs[s][:, 0:D], rec[:])
                        nc.sync.dma_start(
                            out=out_d[b * 512 + s * 128: b * 512 + (s + 1) * 128,
                                      h * D:(h + 1) * D],
                            in_=ot[:])

    nc.compile()
    _prog_cache["nc"] = nc
    return nc


def _host_prep(x, Wq, bq, Wk, bk, Wv, bv):
    """Shard + repack inputs for the 8 cores. Returns in_maps list."""
    assert x.shape == (1, T, C)
    assert np.abs(bq).max() == 0 and np.abs(bk).max() == 0, \
        "nonzero q/k biases not supported"

    x0 = np.ascontiguousarray(x[0]).astype(BF16)
    # xT packed: [128, kc*T + t] = x[t, 128*kc + p]
    xT = np.ascontiguousarray(
        x0.reshape(T, KC, 128).transpose(2, 1, 0).reshape(128, KC * T))

    # causal masks for the 4 diagonal-tile offsets: mask_r[tk, tq] = tq >= tk + 128r
    tq = np.arange(512)[None, :]
    tk = np.arange(128)[:, None]
    masks = np.concatenate(
        [(tq >= (tk + 128 * r)).astype(BF16) for r in range(4)], axis=1)
    masks = np.ascontiguousarray(masks)

    def pack_w(Wrows):
        # Wrows: [128 (out c), C (in)] for one head ->
        # packed[p, 128*kc + c] = Wrows[c, 128*kc + p]
        return np.ascontiguousarray(
            Wrows.astype(BF16).reshape(128, KC, 128).transpose(2, 1, 0)
            .reshape(128, C))

    in_maps = []
    for c in range(N_CORES):
        qheads = [2 * c, 2 * c + 1, 2 * c + 16, 2 * c + 17]
        kvheads = [2 * c, 2 * c + 1]
        wq = np.stack([pack_w(Wq[128 * H:128 * (H + 1)]) for H in qheads])
        wk = np.stack([pack_w(Wk[128 * K:128 * (K + 1)]) for K in kvheads])
        wv = np.stack([pack_w(Wv[128 * K:128 * (K + 1)]) for K in kvheads])
        in_maps.append({
            "xT": xT, "wq": wq, "wk": wk, "wv": wv, "masks": masks,
        })
    return in_maps


def _assemble(results, bv):
    out = np.empty((T, C), dtype=np.float32)
    for c in range(N_CORES):
        r = results[c]["out"]
        qheads = [2 * c, 2 * c + 1, 2 * c + 16, 2 * c + 17]
        for i, H in enumerate(qheads):
            blk = r[:, 128 * i:128 * (i + 1)]
            if bv is not None:
                blk = blk + bv[128 * (H % N_KV_IDX):128 * (H % N_KV_IDX) + 128]
            out[:, 128 * H:128 * (H + 1)] = blk
    return out.reshape(1, T, C)


N_KV_IDX = 16


def _install_trace_hooks():
    """The agent image's antenv lacks axon_hooks; recreate it so
    run_bass_kernel_spmd's trace=True path can capture NTFF profiles."""
    import sys
    import types
    import antenv
    if "antenv.axon_hooks" not in sys.modules:
        mod = types.ModuleType("antenv.axon_hooks")
        mod._hook = None

        def set_axon_ntff_profile_hook(h):
            mod._hook = h

        def get_axon_ntff_profile_hook():
            return mod._hook

        mod.set_axon_ntff_profile_hook = set_axon_ntff_profile_hook
        mod.get_axon_ntff_profile_hook = get_axon_ntff_profile_hook
        sys.modules["antenv.axon_hooks"] = mod
        antenv.axon_hooks = mod
    from antenv.axon_hooks import (get_axon_ntff_profile_hook,
                                   set_axon_ntff_profile_hook)
    if get_axon_ntff_profile_hook() is None:
        if "/root/.axon_site" not in sys.path:
            sys.path.insert(0, "/root/.axon_site")
        from trn_agent_boot.trn_boot import _ntff_profile_via_ctypes
        set_axon_ntff_profile_hook(
            _ntff_profile_via_ctypes("/opt/axon/libaxon_pjrt.so"))
    import concourse.bass_utils as bu
    bu.upload_artifacts = lambda tmpdir: tmpdir


def _run(inputs, trace=False, trace_kwargs=None):
    if trace:
        _install_trace_hooks()
    from concourse.bass_utils import run_bass_kernel_spmd
    nc = _build_program()
    in_maps = _host_prep(**inputs)
    res = run_bass_kernel_spmd(
        nc, in_maps, list(range(N_CORES)), trace=trace,
        **(trace_kwargs or {}))
    bv = inputs["bv"].astype(np.float32)
    bv = bv if np.abs(bv).max() > 0 else None
    out = _assemble(res.results, bv)
    return out, res


def kernel(x, Wq, bq, Wk, bk, Wv, bv):
    out, _ = _run(dict(x=np.asarray(x), Wq=np.asarray(Wq), bq=np.asarray(bq),
                       Wk=np.asarray(Wk), bk=np.asarray(bk),
                       Wv=np.asarray(Wv), bv=np.asarray(bv)))
    return out


# revision 6
# speedup vs baseline: 1.1019x; 1.1019x over previous
"""GQA kernel for Trainium2, 8 NeuronCores, tensor-parallel over heads.

Problem: B=1, T=2048, C=4096, 32 q-heads, 16 kv-heads, head_dim=128,
scale = 1/sqrt(32), causal. q head H uses kv head H%16.

Sharding (no collectives needed): core c owns q-heads
{2c, 2c+1, 2c+16, 2c+17} and kv-heads {2c, 2c+1}. Each output column
block depends only on its own head, so the full output is a host-side
concat of per-core column slices.

Per-core kernel (all matmuls bf16, fp32 PSUM accumulation):
  xT resident in SBUF as [C=4096 (32 chunks of 128 part), T=2048].
  qT[h] = Wq_h @ xT    -> [128 (D), 2048 (T)]   (4 heads)
  kT[kv] = Wk_kv @ xT  -> [128 (D), 2048 (T)]   (2 kv heads)
  v[kv]  = x @ Wv_kv^T -> [2048 (T part), 128+1] (ones col for row sums)
  S^T tile = kT_chunk^T @ qT_block -> [128 Tk, 512 Tq] PSUM
  P^T = exp(SCALE * S^T) (ACT, no max subtraction -- logits are O(+-16),
        safe in fp32/bf16), causal mask via {0,1} multiply on diag tiles.
  out[Tq,128] (+ row-sum col) = sum_j P^T_j.T @ v_j  (PSUM accum)
  out normalized by reciprocal(row sum) (DVE), DMA'd out fp32.
"""

import numpy as np
import ml_dtypes

BF16 = ml_dtypes.bfloat16
T = 2048
C = 4096
D = 128
N_HEADS = 32
N_KV = 16
SCALE = float(1.0 / np.sqrt(np.float32(N_HEADS)))
KC = C // 128          # 32 contraction chunks
NQH = 4                # local q heads per core
NKV = 2                # local kv heads per core
NT = T // 128          # 16 token tiles
VROW = D + 1           # 129: v with ones column
N_CORES = 8

_prog_cache = {}


def _build_program():
    if "nc" in _prog_cache:
        return _prog_cache["nc"]
    import concourse.bass as bass
    import concourse.tile as tile
    from concourse import bacc, mybir

    dt = mybir.dt
    f32 = dt.float32
    bf16 = dt.bfloat16
    EXP = mybir.ActivationFunctionType.Exp

    nc = bacc.Bacc("TRN2", target_bir_lowering=False, debug=False,
                   num_devices=N_CORES)

    xT_d = nc.dram_tensor("xT", [128, KC * T], bf16, kind="ExternalInput").ap()
    wq_d = nc.dram_tensor("wq", [NQH, 128, C], bf16, kind="ExternalInput").ap()
    wk_d = nc.dram_tensor("wk", [NKV, 128, C], bf16, kind="ExternalInput").ap()
    wv_d = nc.dram_tensor("wv", [NKV, 128, C], bf16, kind="ExternalInput").ap()
    # masks: 4x [128,512] causal tiles + [128,128] identity for PE transpose
    mask_d = nc.dram_tensor("masks", [128, 4 * 512 + 128], bf16,
                            kind="ExternalInput").ap()
    out_d = nc.dram_tensor("out", [T, NQH * D], f32, kind="ExternalOutput").ap()

    with tile.TileContext(nc) as tc:
        with (
            tc.tile_pool(name="persist", bufs=1) as persist,
            tc.tile_pool(name="xpool", bufs=1) as xpool,
            tc.tile_pool(name="wpool", bufs=2) as wpool,
            tc.tile_pool(name="ptpool", bufs=3) as ptpool,
            tc.tile_pool(name="opool", bufs=4) as opool,
            tc.tile_pool(name="recpool", bufs=4) as recpool,
            tc.tile_pool(name="psum", bufs=8, space=bass.MemorySpace.PSUM) as psum,
        ):
            mask_sb = persist.tile([128, 4 * 512 + 128], bf16, name="mask_sb",
                                   tag="mask_sb")
            nc.sync.dma_start(out=mask_sb[:], in_=mask_d[:])
            ident = mask_sb[:, 4 * 512: 4 * 512 + 128]

            qt = persist.tile([128, NQH * T], bf16, name="qt", tag="qt")
            kt = persist.tile([128, NKV * T], bf16, name="kt", tag="kt")
            vt = persist.tile([128, NKV * NT * VROW], bf16, name="vt", tag="vt")

            # ones columns of v (row-sum trick)
            for i in range(NKV * NT):
                nc.vector.memset(vt[:, i * VROW + D: (i + 1) * VROW], 1.0)

            # DMA order: first unit's weights, then xT groups interleaved with
            # the remaining units' weights (weight tiles are the first gating
            # dependency for each unit's matmuls).
            xts = [None] * 4
            wts = {}

            def dma_w(src, idx, key):
                w = wpool.tile([128, C], bf16, name=f"w_{key}", tag="w")
                nc.sync.dma_start(out=w[:], in_=src[idx])
                wts[key] = w

            def dma_x(g):
                xt = xpool.tile([128, 8 * T], bf16, name=f"xt{g}", tag=f"xt{g}")
                nc.sync.dma_start(out=xt[:], in_=xT_d[:, g * 8 * T:(g + 1) * 8 * T])
                xts[g] = xt

            def xs(kc, lo, size):
                g, off = divmod(kc, 8)
                return xts[g][:, off * T + lo: off * T + lo + size]

            def proj_dt(key, dest, dbase):
                """[D, T] projection strip: dest[:, dbase:dbase+T] = W @ xT."""
                w = wts.pop(key)
                with nc.named_scope(f"proj_{key}"):
                    for t4 in range(4):
                        ps = psum.tile([128, 512], f32, name=f"ps_{key}_{t4}",
                                       tag="ps")
                        for kc in range(KC):
                            nc.tensor.matmul(
                                ps[:],
                                lhsT=w[:, kc * 128:(kc + 1) * 128],
                                rhs=xs(kc, t4 * 512, 512),
                                start=(kc == 0), stop=(kc == KC - 1),
                            )
                        nc.vector.tensor_copy(
                            out=dest[:, dbase + t4 * 512: dbase + (t4 + 1) * 512],
                            in_=ps[:])

            def proj_v(kv):
                """v[T,D] for kv head: compute vT strip [D,T] (N=512 streams),
                then PE-transpose 128x128 tiles into vt."""
                vts = wpool.tile([128, T], bf16, name=f"vts_{kv}", tag="w")
                proj_dt(f"v{kv}", vts, 0)
                with nc.named_scope(f"vtr_{kv}"):
                    for m in range(NT):
                        ps = psum.tile([128, 128], bf16, name=f"ps_vt_{kv}_{m}",
                                       tag="ps")
                        nc.tensor.transpose(
                            ps[:], vts[:, m * 128:(m + 1) * 128], ident)
                        nc.vector.tensor_copy(
                            out=vt[:, (kv * NT + m) * VROW:
                                   (kv * NT + m) * VROW + D],
                            in_=ps[:])

            def attn(h):
                kv = h % 2
                with nc.named_scope(f"attn_{h}"):
                    for b in range(4):  # Tq blocks of 512
                        pvs = []
                        for s in range(4):
                            pv = psum.tile([128, 512], f32,
                                           name=f"pv_{h}_{b}_{s}", tag="ps")
                            pvs.append(pv)
                        for j in range(4 * b + 4):  # Tk tiles of 128
                            sp = psum.tile([128, 512], f32,
                                           name=f"sp_{h}_{b}_{j}", tag="ps")
                            nc.tensor.matmul(
                                sp[:],
                                lhsT=kt[:, kv * T + j * 128: kv * T + (j + 1) * 128],
                                rhs=qt[:, h * T + b * 512: h * T + (b + 1) * 512],
                                start=True, stop=True,
                            )
                            pt = ptpool.tile([128, 512], bf16,
                                             name=f"pt_{h}_{b}_{j}", tag="pt")
                            nc.scalar.activation(pt[:], sp[:], EXP, scale=SCALE)
                            r = j - 4 * b
                            if r >= 0:
                                nc.vector.tensor_mul(
                                    pt[:], pt[:],
                                    mask_sb[:, r * 512:(r + 1) * 512])
                            vsl = vt[:, (kv * NT + j) * VROW:
                                     (kv * NT + j + 1) * VROW]
                            for s in range(max(0, r), 4):
                                nc.tensor.matmul(
                                    pvs[s][:, 0:VROW],
                                    lhsT=pt[:, s * 128:(s + 1) * 128],
                                    rhs=vsl,
                                    start=(j == 0), stop=(j == 4 * b + s),
                                )
                        for s in range(4):
                            rec = recpool.tile([128, 1], f32,
                                               name=f"rec_{h}_{b}_{s}", tag="rec")
                            nc.vector.reciprocal(rec[:], pvs[s][:, D:D + 1])
                            ot = opool.tile([128, 128], f32,
                                            name=f"ot_{h}_{b}_{s}", tag="ot")
                            nc.vector.tensor_scalar_mul(ot[:], pvs[s][:, 0:D],
                                                        rec[:])
                            nc.sync.dma_start(
                                out=out_d[b * 512 + s * 128:
                                          b * 512 + (s + 1) * 128,
                                          h * D:(h + 1) * D],
                                in_=ot[:])

            # DMA schedule: first unit weights early, xT groups next,
            # remaining weights as consumed.
            dma_w(wq_d, 0, "q0")
            dma_x(0)
            dma_w(wk_d, 0, "k0")
            dma_x(1)
            dma_x(2)
            dma_x(3)
            dma_w(wv_d, 0, "v0")

            # unit/attention interleave; local head -> kv: h%2.
            # order: q0 k0 v0 [h0] q2 [h2] q1 k1 v1 [h1] q3 [h3]
            proj_dt("q0", qt, 0)
            proj_dt("k0", kt, 0)
            proj_v(0)
            attn(0)
            dma_w(wq_d, 2, "q2")
            proj_dt("q2", qt, 2 * T)
            attn(2)
            dma_w(wq_d, 1, "q1")
            proj_dt("q1", qt, T)
            dma_w(wk_d, 1, "k1")
            proj_dt("k1", kt, T)
            dma_w(wv_d, 1, "v1")
            proj_v(1)
            attn(1)
            dma_w(wq_d, 3, "q3")
            proj_dt("q3", qt, 3 * T)
            attn(3)

    nc.compile()
    _prog_cache["nc"] = nc
    return nc


def _host_prep(x, Wq, bq, Wk, bk, Wv, bv):
    """Shard + repack inputs for the 8 cores. Returns in_maps list."""
    assert x.shape == (1, T, C)
    assert np.abs(bq).max() == 0 and np.abs(bk).max() == 0, \
        "nonzero q/k biases not supported"

    x0 = np.ascontiguousarray(x[0]).astype(BF16)
    # xT packed: [128, kc*T + t] = x[t, 128*kc + p]
    xT = np.ascontiguousarray(
        x0.reshape(T, KC, 128).transpose(2, 1, 0).reshape(128, KC * T))

    # causal masks for the 4 diagonal-tile offsets: mask_r[tk, tq] = tq >= tk + 128r
    tq = np.arange(512)[None, :]
    tk = np.arange(128)[:, None]
    masks = np.concatenate(
        [(tq >= (tk + 128 * r)).astype(BF16) for r in range(4)]
        + [np.eye(128, dtype=BF16)], axis=1)
    masks = np.ascontiguousarray(masks)

    def pack_w(Wrows):
        # Wrows: [128 (out c), C (in)] for one head ->
        # packed[p, 128*kc + c] = Wrows[c, 128*kc + p]
        return np.ascontiguousarray(
            Wrows.astype(BF16).reshape(128, KC, 128).transpose(2, 1, 0)
            .reshape(128, C))

    in_maps = []
    for c in range(N_CORES):
        qheads = [2 * c, 2 * c + 1, 2 * c + 16, 2 * c + 17]
        kvheads = [2 * c, 2 * c + 1]
        wq = np.stack([pack_w(Wq[128 * H:128 * (H + 1)]) for H in qheads])
        wk = np.stack([pack_w(Wk[128 * K:128 * (K + 1)]) for K in kvheads])
        wv = np.stack([pack_w(Wv[128 * K:128 * (K + 1)]) for K in kvheads])
        in_maps.append({
            "xT": xT, "wq": wq, "wk": wk, "wv": wv, "masks": masks,
        })
    return in_maps


def _assemble(results, bv):
    out = np.empty((T, C), dtype=np.float32)
    for c in range(N_CORES):
        r = results[c]["out"]
        qheads = [2 * c, 2 * c + 1, 2 * c + 16, 2 * c + 17]
        for i, H in enumerate(qheads):
            blk = r[:, 128 * i:128 * (i + 1)]
            if bv is not None:
                blk = blk + bv[128 * (H % N_KV_IDX):128 * (H % N_KV_IDX) + 128]
            out[:, 128 * H:128 * (H + 1)] = blk
    return out.reshape(1, T, C)


N_KV_IDX = 16


def _install_trace_hooks():
    """The agent image's antenv lacks axon_hooks; recreate it so
    run_bass_kernel_spmd's trace=True path can capture NTFF profiles."""
    import sys
    import types
    import antenv
    if "antenv.axon_hooks" not in sys.modules:
        mod = types.ModuleType("antenv.axon_hooks")
        mod._hook = None

        def set_axon_ntff_profile_hook(h):
            mod._hook = h

        def get_axon_ntff_profile_hook():
            return mod._hook

        mod.set_axon_ntff_profile_hook = set_axon_ntff_profile_hook
        mod.get_axon_ntff_profile_hook = get_axon_ntff_profile_hook
        sys.modules["antenv.axon_hooks"] = mod
        antenv.axon_hooks = mod
    from antenv.axon_hooks import (get_axon_ntff_profile_hook,
                                   set_axon_ntff_profile_hook)
    if get_axon_ntff_profile_hook() is None:
        if "/root/.axon_site" not in sys.path:
            sys.path.insert(0, "/root/.axon_site")
        from trn_agent_boot.trn_boot import _ntff_profile_via_ctypes
        set_axon_ntff_profile_hook(
            _ntff_profile_via_ctypes("/opt/axon/libaxon_pjrt.so"))
    import concourse.bass_utils as bu
    bu.upload_artifacts = lambda tmpdir: tmpdir


def _run(inputs, trace=False, trace_kwargs=None):
    if trace:
        _install_trace_hooks()
    from concourse.bass_utils import run_bass_kernel_spmd
    nc = _build_program()
    in_maps = _host_prep(**inputs)
    res = run_bass_kernel_spmd(
        nc, in_maps, list(range(N_CORES)), trace=trace,
        **(trace_kwargs or {}))
    bv = inputs["bv"].astype(np.float32)
    bv = bv if np.abs(bv).max() > 0 else None
    out = _assemble(res.results, bv)
    return out, res


def kernel(x, Wq, bq, Wk, bk, Wv, bv):
    out, _ = _run(dict(x=np.asarray(x), Wq=np.asarray(Wq), bq=np.asarray(bq),
                       Wk=np.asarray(Wk), bk=np.asarray(bk),
                       Wv=np.asarray(Wv), bv=np.asarray(bv)))
    return out
